# revision 17
# baseline (speedup 1.0000x reference)
"""Trainium2 Bass kernel for ContextWindowPredictor.

Computation (per batch b):
    e1 = hidden[b][pairs[b,:,0]]          # (P, H) gather
    e2 = hidden[b][pairs[b,:,1]]          # (P, H) gather
    h  = gelu([e1 e2] @ W1 + b1)          # (P, H)
    out = h @ W2 + b2                     # (P, 2)

Sharding: data-parallel over batch, one batch per NeuronCore (8 cores).

Device strategy (per core) — token-factored ("U/V") formulation:
    h[p] = gelu(U[s0_p] + V[s1_p])   with U = hid @ W1[:H] + b1,
                                          V = hid @ W1[H:]
Only S=2048 distinct tokens feed P=4096 pairs, so precomputing U,V costs
half the FLOPs of the direct per-pair matmul.

Pipeline (everything stays in SBUF, no DRAM scratch):
  prologue: chunked HWDGE fp32 loads of hid/W1 (double-buffered staging),
            PE transposes straight off the fp32 staging into PSUM, with
            the PSUM->SBUF copy converting to bf16:
            hidT[q, j, s] = hid[s, j*128+q]. W1 converted on ACT into
            w1sb[q, piece, j, m] = W1[piece*H + j*128 + q, m].
  stage 1:  U,V token precompute on the PE in four source-phases
            U0, V0, U1, V1 (h'-halves of U then V), each 16 token-tiles
            x 8 k-tile matmuls; b1 folded into the U PSUM->SBUF copy on
            DVE, V copied on ACT. Output [128, st, h'-half]: token
            st*128+q on partition q. U1/V1 reuse U0/V0's SBUF (tag pool).
  stage 2:  SBUF-source transposed dma_gather (tokens_per_rank=128,
            4 SWDGE queues) yields e1T/e2T tiles [128, j, pair] with
            h' = j*128+q on partitions; gathers for a source chase the
            end of its phase, overlapping later phases' matmuls.
            h = gelu(e1T+e2T) via DVE add + ACT gelu; the W2 contraction
            runs on the PE with W2 k-tiles [128, 2] stationary giving
            logits.T [2, pair] in PSUM. b2 folded into the half-0 PSUM
            readout (ACT Identity bias); half-1 accumulated in place with
            a DVE add. Output [2, P] (SWDGE cast-DMA to fp32); host
            transposes.
"""

import sys

import numpy as np

if "/opt/trn_rl_repo" not in sys.path:
    sys.path.insert(0, "/opt/trn_rl_repo")

B, S, H, P = 8, 2048, 1024, 4096
N_CORES = 8
NI = 512             # pairs per gather chunk
NCH = P // NI        # chunks
ST = S // 128        # token tiles
JT = H // 128        # k-tiles over input h
HH = H // 2          # h' half width
JH = HH // 128       # k-tiles over h' per half
HC = 4               # hid load chunks (4 token-tiles each)
NQ = 1               # SWDGE queues for gathers

_CACHE: dict = {}


def _build():
    import concourse.bacc as bacc
    import concourse.mybir as mybir
    from concourse.masks import make_identity
    from concourse.tile import TileContext

    dt = mybir.dt
    AF = mybir.ActivationFunctionType

    nc = bacc.Bacc("TRN2", target_bir_lowering=False, num_swdge_queues=NQ)

    hid = nc.dram_tensor("hid", [S, H], dt.float32, kind="ExternalInput")
    idx0 = nc.dram_tensor("idx0", [128, P // 16], dt.int16, kind="ExternalInput")
    idx1 = nc.dram_tensor("idx1", [128, P // 16], dt.int16, kind="ExternalInput")
    w1 = nc.dram_tensor("w1", [2 * H, H], dt.float32, kind="ExternalInput")
    b1r = nc.dram_tensor("b1r", [128, H], dt.float32, kind="ExternalInput")
    w2f = nc.dram_tensor("w2f", [128, JT, 2], dt.float32, kind="ExternalInput")
    b2c = nc.dram_tensor("b2c", [2, 1], dt.float32, kind="ExternalInput")
    outN = nc.dram_tensor("outN", [2, P], dt.float32, kind="ExternalOutput")

    with TileContext(nc) as tc:
        with tc.tile_pool(name="const", bufs=1) as cpool:
            i0s = cpool.tile([128, P // 16], dt.int16, tag="i0s")
            nc.sync.dma_start(out=i0s[:], in_=idx0[:])
            i1s = cpool.tile([128, P // 16], dt.int16, tag="i1s")
            nc.sync.dma_start(out=i1s[:], in_=idx1[:])
            b1s = cpool.tile([128, H], dt.bfloat16, tag="b1s")
            w2s = cpool.tile([128, JT, 2], dt.bfloat16, tag="w2s")
            b2s = cpool.tile([2, 1], dt.float32, tag="b2s")
            nc.sync.dma_start(out=b2s[:], in_=b2c[:])

            lg0 = cpool.tile([2, NCH, NI], dt.bfloat16, tag="lg0")
            hidT = cpool.tile([128, JT, S], dt.bfloat16, tag="hidT")
            w1sb = cpool.tile([128, 2, JT, H], dt.bfloat16, tag="w1sb")

            ident = cpool.tile([128, 128], dt.float32, tag="ident")
            make_identity(nc, ident[:])

            # ---- prologue: chunked fp32 loads + PE transposes + converts ----
            with (
                tc.tile_pool(name="hb", bufs=2) as hbpool,
                tc.tile_pool(name="pst", bufs=2, space="PSUM") as tpool,
            ):
                b1f32 = hbpool.tile([128, H], dt.float32, tag="b1f32", bufs=1)
                nc.sync.dma_start(out=b1f32[:], in_=b1r[:])
                nc.vector.tensor_copy(b1s[:], b1f32[:])
                w2s32 = hbpool.tile([128, JT, 2], dt.float32, tag="w2s32", bufs=1)
                nc.sync.dma_start(out=w2s32[:], in_=w2f[:])
                nc.vector.tensor_copy(w2s[:], w2s32[:])

                spt = ST // HC  # token tiles per hid chunk

                def load_hid_chunk(k):
                    h32 = hbpool.tile(
                        [128, spt, H], dt.float32, tag="h32", name=f"h32_{k}", bufs=3
                    )
                    nc.sync.dma_start(
                        out=h32[:],
                        in_=hid[k * spt * 128 : (k + 1) * spt * 128, :].rearrange(
                            "(st p) h -> p st h", p=128
                        ),
                    )
                    return h32

                def load_w1_chunk(piece, mh):
                    wst = hbpool.tile(
                        [128, JT, HH], dt.float32, tag="wst", name=f"wst_{piece}_{mh}"
                    )
                    nc.sync.dma_start(
                        out=wst[:],
                        in_=w1[
                            piece * H : (piece + 1) * H, mh * HH : (mh + 1) * HH
                        ].rearrange("(j p) m -> p j m", p=128),
                    )
                    nc.scalar.activation(
                        w1sb[:, piece, :, mh * HH : (mh + 1) * HH], wst[:], AF.Copy
                    )

                def transpose_chunk(k, h32):
                    for sl in range(spt):
                        st = k * spt + sl
                        for jg in range(2):
                            tp = tpool.tile([128, 512], dt.float32, tag="tp")
                            for jj in range(4):
                                j = jg * 4 + jj
                                nc.tensor.transpose(
                                    tp[:, jj * 128 : (jj + 1) * 128],
                                    h32[:, sl, j * 128 : (j + 1) * 128],
                                    ident[:],
                                )
                            dst = hidT[
                                :, jg * 4 : (jg + 1) * 4, st * 128 : (st + 1) * 128
                            ]
                            if (st + jg) % 2 == 0:
                                nc.vector.tensor_copy(dst, tp[:])
                            else:
                                nc.scalar.activation(dst, tp[:], AF.Copy)

                h32_0 = load_hid_chunk(0)
                load_w1_chunk(0, 0)  # U-half, m0 — needed first for phase U0
                h32_1 = load_hid_chunk(1)
                h32_2 = load_hid_chunk(2)
                transpose_chunk(0, h32_0)
                h32_3 = load_hid_chunk(3)
                transpose_chunk(1, h32_1)
                load_w1_chunk(0, 1)  # U-half, m1
                transpose_chunk(2, h32_2)
                load_w1_chunk(1, 0)  # V-half, m0
                transpose_chunk(3, h32_3)
                load_w1_chunk(1, 1)  # V-half, m1

            # ---- stage 1 + stage 2, pipelined over source phases ----
            with (
                tc.tile_pool(name="uv", bufs=1) as uvpool,
                tc.tile_pool(name="g1", bufs=8) as g1pool,
                tc.tile_pool(name="g2", bufs=5) as g2pool,
                tc.tile_pool(name="hp", bufs=2) as hppool,
                tc.tile_pool(name="ha", bufs=3) as happool,
                tc.tile_pool(name="ps1", bufs=6, space="PSUM") as ps1,
                tc.tile_pool(name="psw", bufs=2, space="PSUM") as psw,
            ):
                uv: dict = {}
                gq = [0]

                def stage1_phase(piece, q):
                    src = uvpool.tile(
                        [128, ST, HH],
                        dt.bfloat16,
                        tag="UV"[piece],
                        name=f"{'UV'[piece]}{q}",
                    )
                    uv[(piece, q)] = src
                    hsl = slice(q * HH, (q + 1) * HH)
                    for st in range(ST):
                        ps = ps1.tile([128, HH], dt.float32, tag="ps")
                        for j in range(JT):
                            nc.tensor.matmul(
                                ps[:],
                                hidT[:, j, st * 128 : (st + 1) * 128],
                                w1sb[:, piece, j, hsl],
                                start=(j == 0),
                                stop=(j == JT - 1),
                            )
                        if piece == 0:
                            nc.vector.tensor_add(src[:, st, :], ps[:], b1s[:, hsl])
                        else:
                            nc.scalar.activation(src[:, st, :], ps[:], AF.Copy)
                        yield st

                def run_phase(piece, q):
                    for _ in stage1_phase(piece, q):
                        pass

                def gather(ei, isrc, src, c):
                    csl = slice(c * (NI // 16), (c + 1) * (NI // 16))
                    nc.gpsimd.dma_gather(
                        out_ap=ei[:],
                        in_ap=src[:],
                        idxs_ap=isrc[:, csl],
                        num_idxs=NI,
                        num_idxs_reg=NI,
                        elem_size=HH,
                        transpose=True,
                        sbuf_tokens_per_rank=128,
                        sbuf_free_dim_per_rank=HH * 2,
                        sbuf_free_dim_pad_per_rank=0,
                        sbuf_byte_offset=0,
                        queue_num=gq[0] % NQ,
                    )
                    gq[0] += 1

                e1t: dict = {}
                e2t: dict = {}

                def alloc_e1(q, c):
                    e1t[(q, c)] = g1pool.tile(
                        [128, JH, NI], dt.bfloat16, tag="e1", name=f"e1_{q}_{c}"
                    )
                    gather(e1t[(q, c)], i0s, uv[(0, q)], c)

                def alloc_e2(q, c):
                    e2t[(q, c)] = g2pool.tile(
                        [128, JH, NI], dt.bfloat16, tag="e2", name=f"e2_{q}_{c}"
                    )
                    gather(e2t[(q, c)], i1s, uv[(1, q)], c)

                hat: dict = {}

                def stage2_pre(q, c):
                    """add + gelu for chunk c; ha kept for stage2_post."""
                    e1 = e1t.pop((q, c))
                    e2 = e2t.pop((q, c))
                    hp = hppool.tile([128, JH, NI], dt.bfloat16, tag="hp")
                    nc.vector.tensor_add(hp[:], e1[:], e2[:])
                    ha = happool.tile([128, JH, NI], dt.bfloat16, tag="ha")
                    nc.scalar.activation(ha[:], hp[:], AF.Gelu)
                    hat[(q, c)] = ha

                def stage2_post(q, c):
                    """W2 matmuls + logits readout for chunk c."""
                    ha = hat.pop((q, c))
                    pw = psw.tile([128, NI], dt.float32, tag="pw")
                    for jj in range(JH):
                        nc.tensor.matmul(
                            pw[0:2, :],
                            w2s[:, q * JH + jj, :],
                            ha[:, jj, :],
                            start=(jj == 0),
                            stop=(jj == JH - 1),
                        )
                    if q == 0:
                        nc.scalar.activation(
                            lg0[:, c, :], pw[0:2, :], AF.Identity, bias=b2s[:]
                        )
                    else:
                        # final accumulate in place (element-streamed, safe)
                        nc.vector.tensor_add(lg0[:, c, :], pw[0:2, :], lg0[:, c, :])

                # phase U0; e1(q0) gathers chase it
                run_phase(0, 0)
                for c in range(NCH):
                    alloc_e1(0, c)
                # phase V0; e2(q0) gathers chase it
                run_phase(1, 0)
                for c in range(NCH):
                    alloc_e2(0, c)
                # phase U1, with stage2(q0) c=0..6 interleaved, post one behind
                for st in stage1_phase(0, 1):
                    if st >= 3 and st % 2 == 1:
                        c = (st - 3) // 2
                        stage2_pre(0, c)
                        if c >= 1:
                            stage2_post(0, c - 1)
                # e1(q1) gathers chase U1
                for c in range(NCH):
                    alloc_e1(1, c)
                # phase V1, with the last q0 chunks early
                for st in stage1_phase(1, 1):
                    if st == 1:
                        stage2_pre(0, NCH - 1)
                        stage2_post(0, NCH - 2)
                    elif st == 3:
                        stage2_post(0, NCH - 1)
                # e2(q1) gathers chase V1
                for c in range(NCH):
                    alloc_e2(1, c)
                # stage2(q1), software-pipelined: pre runs one chunk ahead
                stage2_pre(1, 0)
                for c in range(1, NCH):
                    stage2_pre(1, c)
                    stage2_post(1, c - 1)
                stage2_post(1, NCH - 1)

                nc.gpsimd.dma_start(out=outN[:], in_=lg0[:])

    nc.compile()
    return nc


def _get_nc():
    if "nc" not in _CACHE:
        _CACHE["nc"] = _build()
    return _CACHE["nc"]


def _wrap_idx(idx: np.ndarray) -> np.ndarray:
    """Wrap a [P] index list into the [128, P//16] int16 layout dma_gather
    expects: list position i lives at (partition i%16, column i//16),
    replicated across the 8 q7-core partition groups."""
    w = idx.astype(np.int16).reshape(P // 16, 16).T  # [16, P//16]
    return np.ascontiguousarray(np.tile(w, (8, 1)))  # [128, P//16]


def _make_in_maps(hidden_states, pairs, W1, b1, W2, b2):
    hidden_states = np.ascontiguousarray(np.asarray(hidden_states, dtype=np.float32))
    pairs_i = np.asarray(pairs).astype(np.int32)
    W1f = np.ascontiguousarray(np.asarray(W1, dtype=np.float32))
    b1f = np.ascontiguousarray(
        np.broadcast_to(np.asarray(b1, dtype=np.float32).reshape(1, H), (128, H))
    )
    W2p = np.ascontiguousarray(
        np.asarray(W2, dtype=np.float32).reshape(JT, 128, 2).transpose(1, 0, 2)
    )  # [128, j, o] = W2[j*128+q, o]
    b2f = np.ascontiguousarray(np.asarray(b2, dtype=np.float32).reshape(2, 1))
    in_maps = []
    for c in range(N_CORES):
        in_maps.append(
            {
                "hid": hidden_states[c],
                "idx0": _wrap_idx(pairs_i[c, :, 0]),
                "idx1": _wrap_idx(pairs_i[c, :, 1]),
                "w1": W1f,
                "b1r": b1f,
                "w2f": W2p,
                "b2c": b2f,
            }
        )
    return in_maps


def kernel(hidden_states, pairs, W1, b1, W2, b2):
    from concourse.bass_utils import run_bass_kernel_spmd

    nc = _get_nc()
    in_maps = _make_in_maps(hidden_states, pairs, W1, b1, W2, b2)
    res = run_bass_kernel_spmd(nc, in_maps, core_ids=list(range(N_CORES)))
    out = np.stack(
        [
            np.ascontiguousarray(np.asarray(res.results[c]["outN"]).T)
            for c in range(N_CORES)
        ],
        axis=0,
    )
    return out.astype(np.float32)


if __name__ == "__main__":
    rng = np.random.default_rng(0)
    hs = rng.standard_normal((B, S, H), dtype=np.float32)
    pr = rng.integers(0, S, size=(B, P, 2)).astype(np.int32)
    w1_ = (rng.standard_normal((2 * H, H), dtype=np.float32) / np.sqrt(2 * H)).astype(
        np.float32
    )
    b1_ = np.zeros(H, np.float32)
    w2_ = (rng.standard_normal((H, 2), dtype=np.float32) / np.sqrt(H)).astype(
        np.float32
    )
    b2_ = np.zeros(2, np.float32)
    out = kernel(hidden_states=hs, pairs=pr, W1=w1_, b1=b1_, W2=w2_, b2=b2_)
    print("out", out.shape, out.dtype, out[0, :2])


# revision 19
# speedup vs baseline: 1.0649x; 1.0649x over previous
"""Trainium2 Bass kernel for ContextWindowPredictor.

Computation (per batch b):
    e1 = hidden[b][pairs[b,:,0]]          # (P, H) gather
    e2 = hidden[b][pairs[b,:,1]]          # (P, H) gather
    h  = gelu([e1 e2] @ W1 + b1)          # (P, H)
    out = h @ W2 + b2                     # (P, 2)

Sharding: data-parallel over batch, one batch per NeuronCore (8 cores).

Device strategy (per core) — token-factored ("U/V") formulation:
    h[p] = gelu(U[s0_p] + V[s1_p])   with U = hid @ W1[:H] + b1,
                                          V = hid @ W1[H:]
Only S=2048 distinct tokens feed P=4096 pairs, so precomputing U,V costs
half the FLOPs of the direct per-pair matmul.

Pipeline (everything stays in SBUF, no DRAM scratch):
  prologue: chunked HWDGE fp32 loads of hid/W1 (double-buffered staging),
            PE transposes straight off the fp32 staging into PSUM, with
            the PSUM->SBUF copy converting to bf16:
            hidT[q, j, s] = hid[s, j*128+q]. W1 converted on ACT into
            w1sb[q, piece, j, m] = W1[piece*H + j*128 + q, m].
  stage 1:  U,V token precompute on the PE in four source-phases
            U0, V0, U1, V1 (h'-halves of U then V), each 16 token-tiles
            x 8 k-tile matmuls; b1 folded into the U PSUM->SBUF copy on
            DVE, V copied on ACT. Output [128, st, h'-half]: token
            st*128+q on partition q. U1/V1 reuse U0/V0's SBUF (tag pool).
  stage 2:  SBUF-source transposed dma_gather (tokens_per_rank=128,
            4 SWDGE queues) yields e1T/e2T tiles [128, j, pair] with
            h' = j*128+q on partitions; gathers for a source chase the
            end of its phase, overlapping later phases' matmuls.
            h = gelu(e1T+e2T) via DVE add + ACT gelu; the W2 contraction
            runs on the PE with W2 k-tiles [128, 2] stationary giving
            logits.T [2, pair] in PSUM. b2 folded into the half-0 PSUM
            readout (ACT Identity bias); half-1 accumulated in place with
            a DVE add. Output [2, P] (SWDGE cast-DMA to fp32); host
            transposes.
"""

import sys

import numpy as np

if "/opt/trn_rl_repo" not in sys.path:
    sys.path.insert(0, "/opt/trn_rl_repo")

B, S, H, P = 8, 2048, 1024, 4096
N_CORES = 8
NI = 512             # pairs per gather chunk
NCH = P // NI        # chunks
ST = S // 128        # token tiles
JT = H // 128        # k-tiles over input h
HH = H // 2          # h' half width
JH = HH // 128       # k-tiles over h' per half
HC = 4               # hid load chunks (4 token-tiles each)
NQ = 4               # SWDGE queues for gathers

_CACHE: dict = {}


def _build():
    import concourse.bacc as bacc
    import concourse.mybir as mybir
    from concourse.masks import make_identity
    from concourse.tile import TileContext

    dt = mybir.dt
    AF = mybir.ActivationFunctionType

    nc = bacc.Bacc("TRN2", target_bir_lowering=False, num_swdge_queues=NQ)

    hid = nc.dram_tensor("hid", [S, H], dt.float32, kind="ExternalInput")
    idx0 = nc.dram_tensor("idx0", [128, P // 16], dt.int16, kind="ExternalInput")
    idx1 = nc.dram_tensor("idx1", [128, P // 16], dt.int16, kind="ExternalInput")
    w1 = nc.dram_tensor("w1", [2 * H, H], dt.float32, kind="ExternalInput")
    b1r = nc.dram_tensor("b1r", [128, H], dt.float32, kind="ExternalInput")
    w2f = nc.dram_tensor("w2f", [128, JT, 2], dt.float32, kind="ExternalInput")
    b2c = nc.dram_tensor("b2c", [2, 1], dt.float32, kind="ExternalInput")
    outN = nc.dram_tensor("outN", [2, P], dt.float32, kind="ExternalOutput")

    with TileContext(nc) as tc:
        with tc.tile_pool(name="const", bufs=1) as cpool:
            i0s = cpool.tile([128, P // 16], dt.int16, tag="i0s")
            nc.sync.dma_start(out=i0s[:], in_=idx0[:])
            i1s = cpool.tile([128, P // 16], dt.int16, tag="i1s")
            nc.sync.dma_start(out=i1s[:], in_=idx1[:])
            b1s = cpool.tile([128, H], dt.bfloat16, tag="b1s")
            w2s = cpool.tile([128, JT, 2], dt.bfloat16, tag="w2s")
            b2s = cpool.tile([2, 1], dt.float32, tag="b2s")
            nc.sync.dma_start(out=b2s[:], in_=b2c[:])

            lg0 = cpool.tile([2, NCH, NI], dt.bfloat16, tag="lg0")
            hidT = cpool.tile([128, JT, S], dt.bfloat16, tag="hidT")
            w1sb = cpool.tile([128, 2, JT, H], dt.bfloat16, tag="w1sb")

            ident = cpool.tile([128, 128], dt.float32, tag="ident")
            make_identity(nc, ident[:])

            # ---- prologue: chunked fp32 loads + PE transposes + converts ----
            with (
                tc.tile_pool(name="hb", bufs=2) as hbpool,
                tc.tile_pool(name="pst", bufs=2, space="PSUM") as tpool,
            ):
                b1f32 = hbpool.tile([128, H], dt.float32, tag="b1f32", bufs=1)
                nc.sync.dma_start(out=b1f32[:], in_=b1r[:])
                nc.vector.tensor_copy(b1s[:], b1f32[:])
                w2s32 = hbpool.tile([128, JT, 2], dt.float32, tag="w2s32", bufs=1)
                nc.sync.dma_start(out=w2s32[:], in_=w2f[:])
                nc.vector.tensor_copy(w2s[:], w2s32[:])

                spt = ST // HC  # token tiles per hid chunk

                def load_hid_chunk(k):
                    h32 = hbpool.tile(
                        [128, spt, H], dt.float32, tag="h32", name=f"h32_{k}", bufs=3
                    )
                    nc.sync.dma_start(
                        out=h32[:],
                        in_=hid[k * spt * 128 : (k + 1) * spt * 128, :].rearrange(
                            "(st p) h -> p st h", p=128
                        ),
                    )
                    return h32

                def load_w1_chunk(piece, mh):
                    wst = hbpool.tile(
                        [128, JT, HH], dt.float32, tag="wst", name=f"wst_{piece}_{mh}"
                    )
                    nc.sync.dma_start(
                        out=wst[:],
                        in_=w1[
                            piece * H : (piece + 1) * H, mh * HH : (mh + 1) * HH
                        ].rearrange("(j p) m -> p j m", p=128),
                    )
                    nc.scalar.activation(
                        w1sb[:, piece, :, mh * HH : (mh + 1) * HH], wst[:], AF.Copy
                    )

                def transpose_chunk(k, h32):
                    for sl in range(spt):
                        st = k * spt + sl
                        for jg in range(2):
                            tp = tpool.tile([128, 512], dt.float32, tag="tp")
                            for jj in range(4):
                                j = jg * 4 + jj
                                nc.tensor.transpose(
                                    tp[:, jj * 128 : (jj + 1) * 128],
                                    h32[:, sl, j * 128 : (j + 1) * 128],
                                    ident[:],
                                )
                            dst = hidT[
                                :, jg * 4 : (jg + 1) * 4, st * 128 : (st + 1) * 128
                            ]
                            if (st + jg) % 2 == 0:
                                nc.vector.tensor_copy(dst, tp[:])
                            else:
                                nc.scalar.activation(dst, tp[:], AF.Copy)

                h32_0 = load_hid_chunk(0)
                load_w1_chunk(0, 0)  # U-half, m0 — needed first for phase U0
                h32_1 = load_hid_chunk(1)
                h32_2 = load_hid_chunk(2)
                transpose_chunk(0, h32_0)
                h32_3 = load_hid_chunk(3)
                transpose_chunk(1, h32_1)
                load_w1_chunk(0, 1)  # U-half, m1
                transpose_chunk(2, h32_2)
                load_w1_chunk(1, 0)  # V-half, m0
                transpose_chunk(3, h32_3)
                load_w1_chunk(1, 1)  # V-half, m1

            # ---- stage 1 + stage 2, pipelined over source phases ----
            with (
                tc.tile_pool(name="uv", bufs=1) as uvpool,
                tc.tile_pool(name="g1", bufs=7) as g1pool,
                tc.tile_pool(name="g2", bufs=4) as g2pool,
                tc.tile_pool(name="hp", bufs=1) as hppool,
                tc.tile_pool(name="ha", bufs=2) as happool,
                tc.tile_pool(name="ps1", bufs=6, space="PSUM") as ps1,
                tc.tile_pool(name="psw", bufs=2, space="PSUM") as psw,
            ):
                uv: dict = {}
                gq = [0]

                def stage1_phase(piece, q):
                    src = uvpool.tile(
                        [128, ST, HH],
                        dt.bfloat16,
                        tag=f"{'UV'[piece]}{q}",
                        name=f"{'UV'[piece]}{q}",
                    )
                    uv[(piece, q)] = src
                    hsl = slice(q * HH, (q + 1) * HH)
                    for st in range(ST):
                        ps = ps1.tile([128, HH], dt.float32, tag="ps")
                        for j in range(JT):
                            nc.tensor.matmul(
                                ps[:],
                                hidT[:, j, st * 128 : (st + 1) * 128],
                                w1sb[:, piece, j, hsl],
                                start=(j == 0),
                                stop=(j == JT - 1),
                            )
                        if piece == 0:
                            nc.vector.tensor_add(src[:, st, :], ps[:], b1s[:, hsl])
                        else:
                            nc.scalar.activation(src[:, st, :], ps[:], AF.Copy)
                        yield st

                def run_phase(piece, q):
                    for _ in stage1_phase(piece, q):
                        pass

                def gather(ei, isrc, src, c):
                    csl = slice(c * (NI // 16), (c + 1) * (NI // 16))
                    nc.gpsimd.dma_gather(
                        out_ap=ei[:],
                        in_ap=src[:],
                        idxs_ap=isrc[:, csl],
                        num_idxs=NI,
                        num_idxs_reg=NI,
                        elem_size=HH,
                        transpose=True,
                        sbuf_tokens_per_rank=128,
                        sbuf_free_dim_per_rank=HH * 2,
                        sbuf_free_dim_pad_per_rank=0,
                        sbuf_byte_offset=0,
                        queue_num=gq[0] % NQ,
                    )
                    gq[0] += 1

                e1t: dict = {}
                e2t: dict = {}

                def alloc_e1(q, c):
                    e1t[(q, c)] = g1pool.tile(
                        [128, JH, NI], dt.bfloat16, tag="e1", name=f"e1_{q}_{c}"
                    )
                    gather(e1t[(q, c)], i0s, uv[(0, q)], c)

                def alloc_e2(q, c):
                    e2t[(q, c)] = g2pool.tile(
                        [128, JH, NI], dt.bfloat16, tag="e2", name=f"e2_{q}_{c}"
                    )
                    gather(e2t[(q, c)], i1s, uv[(1, q)], c)

                hat: dict = {}

                def stage2_pre(q, c):
                    """add + gelu for chunk c; ha kept for stage2_post."""
                    e1 = e1t.pop((q, c))
                    e2 = e2t.pop((q, c))
                    hp = hppool.tile([128, JH, NI], dt.bfloat16, tag="hp")
                    nc.vector.tensor_add(hp[:], e1[:], e2[:])
                    ha = happool.tile([128, JH, NI], dt.bfloat16, tag="ha")
                    nc.scalar.activation(ha[:], hp[:], AF.Gelu)
                    hat[(q, c)] = ha

                def stage2_post(q, c):
                    """W2 matmuls + logits readout for chunk c."""
                    ha = hat.pop((q, c))
                    pw = psw.tile([128, NI], dt.float32, tag="pw")
                    for jj in range(JH):
                        nc.tensor.matmul(
                            pw[0:2, :],
                            w2s[:, q * JH + jj, :],
                            ha[:, jj, :],
                            start=(jj == 0),
                            stop=(jj == JH - 1),
                        )
                    if q == 0:
                        nc.scalar.activation(
                            lg0[:, c, :], pw[0:2, :], AF.Identity, bias=b2s[:]
                        )
                    else:
                        # final accumulate in place (element-streamed, safe)
                        nc.vector.tensor_add(lg0[:, c, :], pw[0:2, :], lg0[:, c, :])

                # phase U0; e1(q0) gathers chase it (7 slots; c7 deferred)
                run_phase(0, 0)
                for c in range(NCH - 1):
                    alloc_e1(0, c)
                # phase V0; e2(q0) gathers chase it, then the deferred e1 c7
                run_phase(1, 0)
                for c in range(NCH):
                    alloc_e2(0, c)
                alloc_e1(0, NCH - 1)
                # phase U1, with stage2(q0) c=0..6 interleaved, post one behind
                for st in stage1_phase(0, 1):
                    if st >= 3 and st % 2 == 1:
                        c = (st - 3) // 2
                        stage2_pre(0, c)
                        if c >= 1:
                            stage2_post(0, c - 1)
                # e1(q1) gathers c=0..6 chase U1
                for c in range(NCH - 1):
                    alloc_e1(1, c)
                # phase V1, with the last q0 chunks early
                for st in stage1_phase(1, 1):
                    if st == 1:
                        stage2_pre(0, NCH - 1)
                        stage2_post(0, NCH - 2)
                    elif st == 3:
                        stage2_post(0, NCH - 1)
                # e2(q1) gathers chase V1; deadlock-safe order: first 3 e2
                # (fit in free slots), then the deferred e1(q1, 7), then rest
                for c in range(3):
                    alloc_e2(1, c)
                alloc_e1(1, NCH - 1)
                for c in range(3, NCH):
                    alloc_e2(1, c)
                # stage2(q1), software-pipelined: pre runs one chunk ahead
                stage2_pre(1, 0)
                for c in range(1, NCH):
                    stage2_pre(1, c)
                    stage2_post(1, c - 1)
                stage2_post(1, NCH - 1)

                nc.gpsimd.dma_start(out=outN[:], in_=lg0[:])

    nc.compile()
    return nc


def _get_nc():
    if "nc" not in _CACHE:
        _CACHE["nc"] = _build()
    return _CACHE["nc"]


def _wrap_idx(idx: np.ndarray) -> np.ndarray:
    """Wrap a [P] index list into the [128, P//16] int16 layout dma_gather
    expects: list position i lives at (partition i%16, column i//16),
    replicated across the 8 q7-core partition groups."""
    w = idx.astype(np.int16).reshape(P // 16, 16).T  # [16, P//16]
    return np.ascontiguousarray(np.tile(w, (8, 1)))  # [128, P//16]


def _make_in_maps(hidden_states, pairs, W1, b1, W2, b2):
    hidden_states = np.ascontiguousarray(np.asarray(hidden_states, dtype=np.float32))
    pairs_i = np.asarray(pairs).astype(np.int32)
    W1f = np.ascontiguousarray(np.asarray(W1, dtype=np.float32))
    b1f = np.ascontiguousarray(
        np.broadcast_to(np.asarray(b1, dtype=np.float32).reshape(1, H), (128, H))
    )
    W2p = np.ascontiguousarray(
        np.asarray(W2, dtype=np.float32).reshape(JT, 128, 2).transpose(1, 0, 2)
    )  # [128, j, o] = W2[j*128+q, o]
    b2f = np.ascontiguousarray(np.asarray(b2, dtype=np.float32).reshape(2, 1))
    in_maps = []
    for c in range(N_CORES):
        in_maps.append(
            {
                "hid": hidden_states[c],
                "idx0": _wrap_idx(pairs_i[c, :, 0]),
                "idx1": _wrap_idx(pairs_i[c, :, 1]),
                "w1": W1f,
                "b1r": b1f,
                "w2f": W2p,
                "b2c": b2f,
            }
        )
    return in_maps


def kernel(hidden_states, pairs, W1, b1, W2, b2):
    from concourse.bass_utils import run_bass_kernel_spmd

    nc = _get_nc()
    in_maps = _make_in_maps(hidden_states, pairs, W1, b1, W2, b2)
    res = run_bass_kernel_spmd(nc, in_maps, core_ids=list(range(N_CORES)))
    out = np.stack(
        [
            np.ascontiguousarray(np.asarray(res.results[c]["outN"]).T)
            for c in range(N_CORES)
        ],
        axis=0,
    )
    return out.astype(np.float32)


if __name__ == "__main__":
    rng = np.random.default_rng(0)
    hs = rng.standard_normal((B, S, H), dtype=np.float32)
    pr = rng.integers(0, S, size=(B, P, 2)).astype(np.int32)
    w1_ = (rng.standard_normal((2 * H, H), dtype=np.float32) / np.sqrt(2 * H)).astype(
        np.float32
    )
    b1_ = np.zeros(H, np.float32)
    w2_ = (rng.standard_normal((H, 2), dtype=np.float32) / np.sqrt(H)).astype(
        np.float32
    )
    b2_ = np.zeros(2, np.float32)
    out = kernel(hidden_states=hs, pairs=pr, W1=w1_, b1=b1_, W2=w2_, b2=b2_)
    print("out", out.shape, out.dtype, out[0, :2])


# revision 20
# speedup vs baseline: 1.0930x; 1.0264x over previous
"""Trainium2 Bass kernel for ContextWindowPredictor.

Computation (per batch b):
    e1 = hidden[b][pairs[b,:,0]]          # (P, H) gather
    e2 = hidden[b][pairs[b,:,1]]          # (P, H) gather
    h  = gelu([e1 e2] @ W1 + b1)          # (P, H)
    out = h @ W2 + b2                     # (P, 2)

Sharding: data-parallel over batch, one batch per NeuronCore (8 cores).

Device strategy (per core) — token-factored ("U/V") formulation:
    h[p] = gelu(U[s0_p] + V[s1_p])   with U = hid @ W1[:H] + b1,
                                          V = hid @ W1[H:]
Only S=2048 distinct tokens feed P=4096 pairs, so precomputing U,V costs
half the FLOPs of the direct per-pair matmul.

Pipeline (everything stays in SBUF, no DRAM scratch):
  prologue: chunked HWDGE fp32 loads of hid/W1 (double-buffered staging),
            PE transposes straight off the fp32 staging into PSUM, with
            the PSUM->SBUF copy converting to bf16:
            hidT[q, j, s] = hid[s, j*128+q]. W1 converted on ACT into
            w1sb[q, piece, j, m] = W1[piece*H + j*128 + q, m].
  stage 1:  U,V token precompute on the PE in four source-phases
            U0, V0, U1, V1 (h'-halves of U then V), each 16 token-tiles
            x 8 k-tile matmuls; b1 folded into the U PSUM->SBUF copy on
            DVE, V copied on ACT. Output [128, st, h'-half]: token
            st*128+q on partition q. U1/V1 reuse U0/V0's SBUF (tag pool).
  stage 2:  SBUF-source transposed dma_gather (tokens_per_rank=128,
            4 SWDGE queues) yields e1T/e2T tiles [128, j, pair] with
            h' = j*128+q on partitions; gathers for a source chase the
            end of its phase, overlapping later phases' matmuls.
            h = gelu(e1T+e2T) via DVE add + ACT gelu; the W2 contraction
            runs on the PE with W2 k-tiles [128, 2] stationary giving
            logits.T [2, pair] in PSUM. b2 folded into the half-0 PSUM
            readout (ACT Identity bias); half-1 accumulated in place with
            a DVE add. Output [2, P] (SWDGE cast-DMA to fp32); host
            transposes.
"""

import sys

import numpy as np

if "/opt/trn_rl_repo" not in sys.path:
    sys.path.insert(0, "/opt/trn_rl_repo")

B, S, H, P = 8, 2048, 1024, 4096
N_CORES = 8
NI = 512             # pairs per gather chunk
NCH = P // NI        # chunks
ST = S // 128        # token tiles
JT = H // 128        # k-tiles over input h
HH = H // 2          # h' half width
JH = HH // 128       # k-tiles over h' per half
HC = 4               # hid load chunks (4 token-tiles each)
NQ = 4               # SWDGE queues for gathers

_CACHE: dict = {}


def _build():
    import concourse.bacc as bacc
    import concourse.mybir as mybir
    from concourse.masks import make_identity
    from concourse.tile import TileContext

    dt = mybir.dt
    AF = mybir.ActivationFunctionType

    nc = bacc.Bacc("TRN2", target_bir_lowering=False, num_swdge_queues=NQ)

    hid = nc.dram_tensor("hid", [S, H], dt.float32, kind="ExternalInput")
    idx0 = nc.dram_tensor("idx0", [128, P // 16], dt.int16, kind="ExternalInput")
    idx1 = nc.dram_tensor("idx1", [128, P // 16], dt.int16, kind="ExternalInput")
    w1 = nc.dram_tensor("w1", [2 * H, H], dt.float32, kind="ExternalInput")
    b1r = nc.dram_tensor("b1r", [128, H], dt.float32, kind="ExternalInput")
    w2f = nc.dram_tensor("w2f", [128, JT, 2], dt.float32, kind="ExternalInput")
    b2c = nc.dram_tensor("b2c", [2, 1], dt.float32, kind="ExternalInput")
    outN = nc.dram_tensor("outN", [2, P], dt.float32, kind="ExternalOutput")

    with TileContext(nc) as tc:
        with tc.tile_pool(name="const", bufs=1) as cpool:
            i0s = cpool.tile([128, P // 16], dt.int16, tag="i0s")
            nc.sync.dma_start(out=i0s[:], in_=idx0[:])
            i1s = cpool.tile([128, P // 16], dt.int16, tag="i1s")
            nc.sync.dma_start(out=i1s[:], in_=idx1[:])
            b1s = cpool.tile([128, H], dt.bfloat16, tag="b1s")
            w2s = cpool.tile([128, JT, 2], dt.bfloat16, tag="w2s")
            b2s = cpool.tile([2, 1], dt.float32, tag="b2s")
            nc.sync.dma_start(out=b2s[:], in_=b2c[:])

            lg0 = cpool.tile([2, NCH, NI], dt.bfloat16, tag="lg0")
            hidT = cpool.tile([128, JT, S], dt.bfloat16, tag="hidT")
            w1sb = cpool.tile([128, 2, JT, H], dt.bfloat16, tag="w1sb")

            ident = cpool.tile([128, 128], dt.float32, tag="ident")
            make_identity(nc, ident[:])

            # ---- prologue: chunked fp32 loads + PE transposes + converts ----
            with (
                tc.tile_pool(name="hb", bufs=2) as hbpool,
                tc.tile_pool(name="pst", bufs=2, space="PSUM") as tpool,
            ):
                b1f32 = hbpool.tile([128, H], dt.float32, tag="b1f32", bufs=1)
                nc.sync.dma_start(out=b1f32[:], in_=b1r[:])
                nc.vector.tensor_copy(b1s[:], b1f32[:])
                w2s32 = hbpool.tile([128, JT, 2], dt.float32, tag="w2s32", bufs=1)
                nc.sync.dma_start(out=w2s32[:], in_=w2f[:])
                nc.vector.tensor_copy(w2s[:], w2s32[:])

                spt = ST // HC  # token tiles per hid chunk

                def load_hid_chunk(k):
                    h32 = hbpool.tile(
                        [128, spt, H], dt.float32, tag="h32", name=f"h32_{k}", bufs=3
                    )
                    nc.sync.dma_start(
                        out=h32[:],
                        in_=hid[k * spt * 128 : (k + 1) * spt * 128, :].rearrange(
                            "(st p) h -> p st h", p=128
                        ),
                    )
                    return h32

                def load_w1_chunk(piece, mh):
                    wst = hbpool.tile(
                        [128, JT, HH], dt.float32, tag="wst", name=f"wst_{piece}_{mh}"
                    )
                    nc.sync.dma_start(
                        out=wst[:],
                        in_=w1[
                            piece * H : (piece + 1) * H, mh * HH : (mh + 1) * HH
                        ].rearrange("(j p) m -> p j m", p=128),
                    )
                    nc.scalar.activation(
                        w1sb[:, piece, :, mh * HH : (mh + 1) * HH], wst[:], AF.Copy
                    )

                def transpose_chunk(k, h32):
                    for sl in range(spt):
                        st = k * spt + sl
                        for jg in range(2):
                            tp = tpool.tile([128, 512], dt.float32, tag="tp")
                            for jj in range(4):
                                j = jg * 4 + jj
                                nc.tensor.transpose(
                                    tp[:, jj * 128 : (jj + 1) * 128],
                                    h32[:, sl, j * 128 : (j + 1) * 128],
                                    ident[:],
                                )
                            dst = hidT[
                                :, jg * 4 : (jg + 1) * 4, st * 128 : (st + 1) * 128
                            ]
                            if (st + jg) % 2 == 0:
                                nc.vector.tensor_copy(dst, tp[:])
                            else:
                                nc.scalar.activation(dst, tp[:], AF.Copy)

                h32_0 = load_hid_chunk(0)
                load_w1_chunk(0, 0)  # U-half, m0 — needed first for phase U0
                h32_1 = load_hid_chunk(1)
                h32_2 = load_hid_chunk(2)
                transpose_chunk(0, h32_0)
                h32_3 = load_hid_chunk(3)
                transpose_chunk(1, h32_1)
                load_w1_chunk(0, 1)  # U-half, m1
                transpose_chunk(2, h32_2)
                load_w1_chunk(1, 0)  # V-half, m0
                transpose_chunk(3, h32_3)
                load_w1_chunk(1, 1)  # V-half, m1

            # ---- stage 1 + stage 2, pipelined over source phases ----
            with (
                tc.tile_pool(name="uv", bufs=1) as uvpool,
                tc.tile_pool(name="g1", bufs=7) as g1pool,
                tc.tile_pool(name="g2", bufs=4) as g2pool,
                tc.tile_pool(name="hp", bufs=2) as hppool,
                tc.tile_pool(name="ha", bufs=2) as happool,
                tc.tile_pool(name="ps1", bufs=6, space="PSUM") as ps1,
                tc.tile_pool(name="psw", bufs=2, space="PSUM") as psw,
            ):
                uv: dict = {}
                gq = [0]

                def stage1_phase(piece, q):
                    src = uvpool.tile(
                        [128, ST, HH],
                        dt.bfloat16,
                        tag=f"{'UV'[piece]}{q}",
                        name=f"{'UV'[piece]}{q}",
                    )
                    uv[(piece, q)] = src
                    hsl = slice(q * HH, (q + 1) * HH)
                    for st in range(ST):
                        ps = ps1.tile([128, HH], dt.float32, tag="ps")
                        for j in range(JT):
                            nc.tensor.matmul(
                                ps[:],
                                hidT[:, j, st * 128 : (st + 1) * 128],
                                w1sb[:, piece, j, hsl],
                                start=(j == 0),
                                stop=(j == JT - 1),
                            )
                        if piece == 0:
                            nc.vector.tensor_add(src[:, st, :], ps[:], b1s[:, hsl])
                        else:
                            nc.scalar.activation(src[:, st, :], ps[:], AF.Copy)
                        yield st

                def run_phase(piece, q):
                    for _ in stage1_phase(piece, q):
                        pass

                def gather(ei, isrc, src, c):
                    csl = slice(c * (NI // 16), (c + 1) * (NI // 16))
                    nc.gpsimd.dma_gather(
                        out_ap=ei[:],
                        in_ap=src[:],
                        idxs_ap=isrc[:, csl],
                        num_idxs=NI,
                        num_idxs_reg=NI,
                        elem_size=HH,
                        transpose=True,
                        sbuf_tokens_per_rank=128,
                        sbuf_free_dim_per_rank=HH * 2,
                        sbuf_free_dim_pad_per_rank=0,
                        sbuf_byte_offset=0,
                        queue_num=gq[0] % NQ,
                    )
                    gq[0] += 1

                e1t: dict = {}
                e2t: dict = {}

                def alloc_e1(q, c):
                    e1t[(q, c)] = g1pool.tile(
                        [128, JH, NI], dt.bfloat16, tag="e1", name=f"e1_{q}_{c}"
                    )
                    gather(e1t[(q, c)], i0s, uv[(0, q)], c)

                def alloc_e2(q, c):
                    e2t[(q, c)] = g2pool.tile(
                        [128, JH, NI], dt.bfloat16, tag="e2", name=f"e2_{q}_{c}"
                    )
                    gather(e2t[(q, c)], i1s, uv[(1, q)], c)

                hat: dict = {}

                def stage2_pre(q, c):
                    """add + gelu for chunk c; ha kept for stage2_post."""
                    e1 = e1t.pop((q, c))
                    e2 = e2t.pop((q, c))
                    hp = hppool.tile([128, JH, NI], dt.bfloat16, tag="hp")
                    nc.vector.tensor_add(hp[:], e1[:], e2[:])
                    ha = happool.tile([128, JH, NI], dt.bfloat16, tag="ha")
                    nc.scalar.activation(ha[:], hp[:], AF.Gelu)
                    hat[(q, c)] = ha

                def stage2_post(q, c):
                    """W2 matmuls + logits readout for chunk c."""
                    ha = hat.pop((q, c))
                    pw = psw.tile([128, NI], dt.float32, tag="pw")
                    for jj in range(JH):
                        nc.tensor.matmul(
                            pw[0:2, :],
                            w2s[:, q * JH + jj, :],
                            ha[:, jj, :],
                            start=(jj == 0),
                            stop=(jj == JH - 1),
                        )
                    if q == 0:
                        nc.scalar.activation(
                            lg0[:, c, :], pw[0:2, :], AF.Identity, bias=b2s[:]
                        )
                    else:
                        # final accumulate in place (element-streamed, safe)
                        nc.vector.tensor_add(lg0[:, c, :], pw[0:2, :], lg0[:, c, :])

                # phase U0; e1(q0) gathers chase it (7 slots; c7 deferred)
                run_phase(0, 0)
                for c in range(NCH - 1):
                    alloc_e1(0, c)
                # phase V0; e2(q0) gathers chase it, then the deferred e1 c7
                run_phase(1, 0)
                for c in range(NCH):
                    alloc_e2(0, c)
                alloc_e1(0, NCH - 1)
                # phase U1, with stage2(q0) c=0..6 interleaved, post one behind
                for st in stage1_phase(0, 1):
                    if st >= 3 and st % 2 == 1:
                        c = (st - 3) // 2
                        stage2_pre(0, c)
                        if c >= 1:
                            stage2_post(0, c - 1)
                # e1(q1) gathers c=0..6 chase U1
                for c in range(NCH - 1):
                    alloc_e1(1, c)
                # phase V1, with the last q0 chunks early
                for st in stage1_phase(1, 1):
                    if st == 1:
                        stage2_pre(0, NCH - 1)
                        stage2_post(0, NCH - 2)
                    elif st == 3:
                        stage2_post(0, NCH - 1)
                # e2(q1) gathers chase V1; deadlock-safe order: first 3 e2
                # (fit in free slots), then the deferred e1(q1, 7), then rest
                for c in range(3):
                    alloc_e2(1, c)
                alloc_e1(1, NCH - 1)
                for c in range(3, NCH):
                    alloc_e2(1, c)
                # stage2(q1), software-pipelined: pre runs one chunk ahead
                stage2_pre(1, 0)
                for c in range(1, NCH):
                    stage2_pre(1, c)
                    stage2_post(1, c - 1)
                stage2_post(1, NCH - 1)

                nc.gpsimd.dma_start(out=outN[:], in_=lg0[:])

    nc.compile()
    return nc


def _get_nc():
    if "nc" not in _CACHE:
        _CACHE["nc"] = _build()
    return _CACHE["nc"]


def _wrap_idx(idx: np.ndarray) -> np.ndarray:
    """Wrap a [P] index list into the [128, P//16] int16 layout dma_gather
    expects: list position i lives at (partition i%16, column i//16),
    replicated across the 8 q7-core partition groups."""
    w = idx.astype(np.int16).reshape(P // 16, 16).T  # [16, P//16]
    return np.ascontiguousarray(np.tile(w, (8, 1)))  # [128, P//16]


def _make_in_maps(hidden_states, pairs, W1, b1, W2, b2):
    hidden_states = np.ascontiguousarray(np.asarray(hidden_states, dtype=np.float32))
    pairs_i = np.asarray(pairs).astype(np.int32)
    W1f = np.ascontiguousarray(np.asarray(W1, dtype=np.float32))
    b1f = np.ascontiguousarray(
        np.broadcast_to(np.asarray(b1, dtype=np.float32).reshape(1, H), (128, H))
    )
    W2p = np.ascontiguousarray(
        np.asarray(W2, dtype=np.float32).reshape(JT, 128, 2).transpose(1, 0, 2)
    )  # [128, j, o] = W2[j*128+q, o]
    b2f = np.ascontiguousarray(np.asarray(b2, dtype=np.float32).reshape(2, 1))
    in_maps = []
    for c in range(N_CORES):
        in_maps.append(
            {
                "hid": hidden_states[c],
                "idx0": _wrap_idx(pairs_i[c, :, 0]),
                "idx1": _wrap_idx(pairs_i[c, :, 1]),
                "w1": W1f,
                "b1r": b1f,
                "w2f": W2p,
                "b2c": b2f,
            }
        )
    return in_maps


def kernel(hidden_states, pairs, W1, b1, W2, b2):
    from concourse.bass_utils import run_bass_kernel_spmd

    nc = _get_nc()
    in_maps = _make_in_maps(hidden_states, pairs, W1, b1, W2, b2)
    res = run_bass_kernel_spmd(nc, in_maps, core_ids=list(range(N_CORES)))
    out = np.stack(
        [
            np.ascontiguousarray(np.asarray(res.results[c]["outN"]).T)
            for c in range(N_CORES)
        ],
        axis=0,
    )
    return out.astype(np.float32)


if __name__ == "__main__":
    rng = np.random.default_rng(0)
    hs = rng.standard_normal((B, S, H), dtype=np.float32)
    pr = rng.integers(0, S, size=(B, P, 2)).astype(np.int32)
    w1_ = (rng.standard_normal((2 * H, H), dtype=np.float32) / np.sqrt(2 * H)).astype(
        np.float32
    )
    b1_ = np.zeros(H, np.float32)
    w2_ = (rng.standard_normal((H, 2), dtype=np.float32) / np.sqrt(H)).astype(
        np.float32
    )
    b2_ = np.zeros(2, np.float32)
    out = kernel(hidden_states=hs, pairs=pr, W1=w1_, b1=b1_, W2=w2_, b2=b2_)
    print("out", out.shape, out.dtype, out[0, :2])
